# revision 47
# baseline (speedup 1.0000x reference)
"""FAVOR+ (Performer) attention kernel for Trainium2, 8 NeuronCores.

Problem: nn_Attention_4810363372688
  B=4, L=4096, HID=1024, H=16, DH=64, M=128, noncausal relu-kernel FAVOR+.

Sharding: core c handles batch b=c//2 and head-group hg=c%2 (8 heads).
Each core computes a partial output (its 8 heads' contribution to the
output projection) in transposed layout [HID, L]; the host sums the two
head-group partials per batch and transposes back. No collectives.

Structure (all per 512-token chunk, software-pipelined):
  pass 1: kT = Wk.T x (transposed), two-stage kp hop via a block-diagonal
          projT pair matmul (relu(x+eps) on ACT), v in [l, hd] layout,
          kv/ks accumulated per head into persistent PSUM banks.
  pass 2: qT, qp (zero-padded K=128 hops), num per (head, l-tile) in
          [l, d] layout with den as column 64 -> reciprocal is a
          per-partition scalar op, att = num * recip via one broadcasted
          DVE mult per 4-head bank, PE-transpose of att pair-tiles feeds
          the output projection. The out-projection of chunk c is emitted
          after chunk c+1's qt/qp so it never stalls at a chunk seam.

Host prep: x pre-chunked to [NCH*128, KT*512] bf16 (8KB contiguous DMA
lines), ratio folded into projT, block-diag / zero-padded projT variants.
"""

import os
import numpy as np
import ml_dtypes

import concourse.bass as bass
import concourse.mybir as mybir
import concourse.tile as tile
from concourse import bacc
from concourse import bass_utils
from concourse.masks import make_identity

B, L, HID, H, DH, M = 4, 4096, 1024, 16, 64, 128
EPS = 1e-3
HC = H // 2          # heads per core = 8
KT = HID // 128      # 8 contraction k-tiles
LT = L // 128        # 32 token tiles (pass 1)
NCH = L // 512       # 8 L-chunks (pass 2)

BF16 = mybir.dt.bfloat16
F32 = mybir.dt.float32

_cache = {}

# exec time of the most recent run (ns), when KERNEL_TRACE=1
last_exec_time_ns = None
last_trace_path = None
last_insts = None
last_res = None


def build_kernel():
    nc = bacc.Bacc("TRN2", target_bir_lowering=False, debug=False, num_devices=1)

    # x inputs are host-pre-chunked: row block ch*128..ch*128+127 holds, for
    # partition p, the KT*512 contiguous values [k, l] of chunk ch -> every
    # DMA line is 8KB contiguous.
    xsT = nc.dram_tensor("xsT", [NCH * 128, KT * 512], BF16, kind="ExternalInput").ap()
    xqT = nc.dram_tensor("xqT", [NCH * 128, KT * 512], BF16, kind="ExternalInput").ap()
    wkv = nc.dram_tensor("wkv", [HID, 2 * HC * DH], BF16, kind="ExternalInput").ap()
    wq = nc.dram_tensor("wq", [HID, HC * DH], BF16, kind="ExternalInput").ap()
    wo = nc.dram_tensor("wo", [HC * DH, HID], BF16, kind="ExternalInput").ap()
    projt = nc.dram_tensor("projt", [DH, M], BF16, kind="ExternalInput").ap()
    projt2 = nc.dram_tensor("projt2", [2 * DH, 2 * M], BF16, kind="ExternalInput").ap()
    outT = nc.dram_tensor("outT", [HID, L], F32, kind="ExternalOutput").ap()

    xsT_c = xsT.rearrange("(c p) (k f) -> c p k f", p=128, f=512)  # [8,128,8,512]
    xqT_c = xqT.rearrange("(c p) (k f) -> c p k f", p=128, f=512)
    wkv_r = wkv.rearrange("(k p) f -> p k f", p=128)     # [128, 8, 1024]
    wq_r = wq.rearrange("(k p) f -> p k f", p=128)       # [128, 8, 512]
    wo_r = wo.rearrange("(t p) o -> p t o", p=128)       # [128, 4, 1024]

    with tile.TileContext(nc) as tc:
        with tc.tile_pool(name="singles", bufs=1) as singles:
            # xs chunk 0 first, split across both DGE rings, so pass 1 can
            # start as early as possible; weights follow
            # interleave the chunk-0 x slices and wkv k-slices across both
            # DGE rings in k order: the first kT accumulation paces with the
            # arrivals instead of waiting for the full 2.5MB
            xs0_sb = singles.tile([128, KT, 512], BF16)
            wkv_sb = singles.tile([128, KT, 2 * HC * DH], BF16)
            # first wave: xs0 slices + just the kT m-tile-0 weight columns
            # (subtile deps let m-tile 0 start streaming ~2-3us earlier);
            # second wave: the remaining wk/wv columns
            for k in range(KT):
                a, b = (nc.sync, nc.scalar) if k % 2 == 0 else (nc.scalar, nc.sync)
                a.dma_start(out=xs0_sb[:, k, :], in_=xsT_c[0, :, k, :])
                b.dma_start(out=wkv_sb[:, k, 0:128], in_=wkv_r[:, k, 0:128])
            for k in range(KT):
                eng = nc.scalar if k % 2 == 0 else nc.sync
                eng.dma_start(out=wkv_sb[:, k, 128:1024], in_=wkv_r[:, k, 128:1024])
            # small tensors next (kp hop needs projt2 within the first chunk)
            # projt: zero-padded per half so qp matmuls run as uniform K=128
            # (dead rows contract against zeros) -> full-rate streaming;
            # projt2 is the block-diagonal pair form (for the kp hop)
            projt_sb = singles.tile([128, 2, M], BF16)
            nc.gpsimd.memset(projt_sb, 0.0)
            nc.scalar.dma_start(out=projt_sb[0:DH, 0, :], in_=projt)
            nc.scalar.dma_start(out=projt_sb[DH:128, 1, :], in_=projt)
            projt2_sb = singles.tile([128, 2 * M], BF16)
            nc.scalar.dma_start(out=projt2_sb, in_=projt2)
            # pass-2 weights queue behind wkv on the scalar ring; they are
            # not needed until pass 2
            wq_sb = singles.tile([128, KT, HC * DH], BF16)
            nc.scalar.dma_start(out=wq_sb, in_=wq_r)
            wo_sb = singles.tile([128, 4, HID], BF16)
            nc.scalar.dma_start(out=wo_sb, in_=wo_r)
            # prefetch pass-2 chunk 0 of xq behind the weights
            xq0_sb = singles.tile([128, KT, 512], BF16)
            nc.scalar.dma_start(out=xq0_sb, in_=xqT_c[0, :, :, :])
            kvsks_sb = singles.tile([128, HC, DH + 1], BF16)
            eps_bias = singles.tile([128, 1], F32)
            nc.vector.memset(eps_bias, EPS)



            # ---------------- pass 1: k -> kp, v -> kvs/ks accumulation ----
            # two-stage K: kT = Wk.T x (transposed, like qT), then the cheap
            # DH->M hop kp = relu(kT.T @ projT + eps') per head-pair via one
            # block-diagonal matmul (full 128-row weights -> FWL).
            with (
                tc.tile_pool(name="p1s", bufs=2) as p1s,
                tc.tile_pool(name="p1ps", bufs=2, space="PSUM") as p1ps,
                tc.tile_pool(name="kvps", bufs=1, space="PSUM") as kvps,
            ):
                kv_ps_lo = kvps.tile([128, 4 * (DH + 1)], F32, tag="kvlo")
                kv_ps_hi = kvps.tile([128, 4 * (DH + 1)], F32, tag="kvhi")

                for ch in range(NCH):
                    if ch == 0:
                        xs = xs0_sb
                    else:
                        xs = p1s.tile([128, KT, 512], BF16, tag="xs", bufs=3)
                        nc.sync.dma_start(out=xs, in_=xsT_c[ch, :, :, :])
                    # kT [hd, l] per m-tile, heads pair-stacked
                    kt_sb = p1s.tile([128, 4, 512], BF16, tag="kt")
                    for t in range(4):
                        kt_ps = p1ps.tile([128, 512], F32, tag="ktp", name="kt_ps")
                        for k in range(KT):
                            nc.tensor.matmul(
                                kt_ps,
                                lhsT=wkv_sb[:, k, t * 128 : (t + 1) * 128],
                                rhs=xs[:, k, :],
                                start=(k == 0), stop=(k == KT - 1),
                            )
                        nc.vector.tensor_copy(out=kt_sb[:, t, :], in_=kt_ps)
                    for lt in range(4):
                        lts = slice(lt * 128, (lt + 1) * 128)
                        v_ps = p1ps.tile([128, 512], F32, tag="vps", name="v_ps")
                        for k in range(KT):
                            nc.tensor.matmul(
                                v_ps,
                                lhsT=xs[:, k, lts],
                                rhs=wkv_sb[:, k, 512:1024],
                                start=(k == 0), stop=(k == KT - 1),
                            )
                        # v with a ones column appended per head: [128, 8, 65]
                        v_sb = p1s.tile([128, HC, DH + 1], BF16, tag="v", bufs=3)
                        nc.scalar.copy(
                            v_sb[:, :, 0:DH],
                            v_ps.rearrange("p (h d) -> p h d", h=HC),
                        )
                        nc.gpsimd.memset(v_sb[:, :, DH : DH + 1], 1.0)
                        # kp hop: per pair-bank, two pair matmuls with the
                        # block-diag projt2 -> [l, 2*M]; relu+eps evac
                        kp_sb = p1s.tile([128, HC, 128], BF16, tag="kp", bufs=3)
                        for gb in range(2):
                            kp_ps = p1ps.tile(
                                [128, 2, 256], F32, tag=f"kpp{gb}", bufs=1,
                                name="kp_ps",
                            )
                            for tp in range(2):
                                t = 2 * gb + tp
                                nc.tensor.matmul(
                                    kp_ps[:, tp, :],
                                    lhsT=kt_sb[:, t, lts],
                                    rhs=projt2_sb,
                                    start=True, stop=True,
                                )
                            # relu(x + eps) ~= relu(x) + eps (err <= eps on a
                            # ~0.04% sliver of inputs) -> single ACT op
                            nc.scalar.activation(
                                kp_sb[:, 4 * gb : 4 * gb + 4, :],
                                kp_ps.rearrange("p a (b m) -> p (a b) m", m=128),
                                mybir.ActivationFunctionType.Relu,
                                bias=eps_bias,
                            )
                        # One accumulation group per PSUM bank: start only on
                        # the very first MM touching the bank.
                        first = ch == 0 and lt == 0
                        last = ch == NCH - 1 and lt == 3
                        for h in range(HC):
                            ps = kv_ps_lo if h < 4 else kv_ps_hi
                            j = h % 4
                            nc.tensor.matmul(
                                ps[:, j * (DH + 1) : (j + 1) * (DH + 1)],
                                lhsT=kp_sb[:, h, :],
                                rhs=v_sb[:, h, :],
                                start=(first and j == 0),
                                stop=(last and j == 3),
                            )

                nc.scalar.copy(
                    kvsks_sb[:, 0:4, :],
                    kv_ps_lo.rearrange("p (h d) -> p h d", h=4),
                )
                nc.vector.tensor_copy(
                    out=kvsks_sb[:, 4:8, :],
                    in_=kv_ps_hi.rearrange("p (h d) -> p h d", h=4),
                )

            # ---------------- pass 2: q -> qp -> num/den -> att -> out ------
            # num is computed per (head, l-tile) in [l, d] layout (lhsT = qp
            # l-tile, rhs = kvs||ks), so den is column 64 -> reciprocal is a
            # per-partition scalar; att = num * recip(den) is one broadcasted
            # DVE mult per 4-head bank. att pair-tiles [l, 128] are then
            # PE-transposed to [hd, l] for the output projection.
            ident = singles.tile([128, 128], BF16)
            make_identity(nc, ident)
            with (
                tc.tile_pool(name="p2s", bufs=2) as p2s,
                tc.tile_pool(name="p2ps", bufs=2, space="PSUM") as p2ps,
            ):
                def emit_out(item):
                    och, oatt = item
                    osl = slice(och * 512, (och + 1) * 512)
                    for j in range(8):
                        out_ps = p2ps.tile(
                            [128, 512], F32, tag="out", bufs=3, name="out_ps"
                        )
                        for t in range(4):
                            nc.tensor.matmul(
                                out_ps,
                                lhsT=wo_sb[:, t, j * 128 : (j + 1) * 128],
                                rhs=oatt[:, t, :],
                                start=(t == 0), stop=(t == 3),
                            )
                        out_sb = p2s.tile(
                            [128, 512], F32, tag="outsb", bufs=3, name="out_sb"
                        )
                        # alternate evac engine and DMA ring by j: halves the
                        # ACT chain latency and drains the tail twice as fast
                        if j % 2 == 0:
                            nc.scalar.copy(out_sb, out_ps)
                            nc.scalar.dma_start(
                                out=outT[j * 128 : (j + 1) * 128, osl], in_=out_sb
                            )
                        else:
                            nc.vector.tensor_copy(out=out_sb, in_=out_ps)
                            nc.sync.dma_start(
                                out=outT[j * 128 : (j + 1) * 128, osl], in_=out_sb
                            )

                xq_next = xq0_sb
                prev = None
                for ch in range(NCH):
                    xq = xq_next
                    if ch + 1 < NCH:
                        xq_next = p2s.tile([128, KT, 512], BF16, tag="xq")
                        nc.sync.dma_start(out=xq_next, in_=xqT_c[ch + 1, :, :, :])
                    # qT [hd, l] per m-tile; heads (2m, 2m+1) stay stacked in
                    # the two partition halves
                    qt_sb = p2s.tile([128, 4, 512], BF16, tag="qt")
                    for m in range(4):
                        qt_ps = p2ps.tile([128, 512], F32, tag="proj", name="qt_ps")
                        for k in range(KT):
                            nc.tensor.matmul(
                                qt_ps,
                                lhsT=wq_sb[:, k, m * 128 : (m + 1) * 128],
                                rhs=xq[:, k, :],
                                start=(k == 0), stop=(k == KT - 1),
                            )
                        nc.scalar.copy(qt_sb[:, m, :], qt_ps)
                    # qp = max(ratio*(q@proj.T), 0) + eps  (ratio folded in
                    # projt); odd heads run at base-64 via row-tiling
                    qp_sb = p2s.tile([128, HC, 512], BF16, tag="qp")
                    for h in range(HC):
                        qp_ps = p2ps.tile([128, 512], F32, tag="proj", name="qp_ps")
                        nc.tensor.matmul(
                            qp_ps,
                            lhsT=projt_sb[:, h % 2, :],
                            rhs=qt_sb[:, h // 2, :],
                            start=True, stop=True,
                        )
                        nc.vector.tensor_scalar(
                            qp_sb[:, h, :], qp_ps, 0.0, EPS,
                            op0=mybir.AluOpType.max, op1=mybir.AluOpType.add,
                        )
                    # out-projection of the PREVIOUS chunk goes here: its
                    # attT evacs complete while this chunk's qt/qp run, so
                    # the out matmuls never stall the PE at a chunk seam.
                    if prev is not None:
                        emit_out(prev)
                    # num/den per l-tile: out [128 l, 65], den in col 64
                    att_sb = p2s.tile([128, 4, 4, 128], BF16, tag="att")
                    oatt = p2s.tile([128, 4, 512], BF16, tag="oatt")

                    def do_transpose(lt):
                        # att pair-tiles [l, 128] -> [hd, l] for the out proj
                        attT_ps = p2ps.tile(
                            [128, 4, 128], BF16, tag="attT", bufs=1,
                            name="attT_ps",
                        )
                        for t in range(4):
                            nc.tensor.transpose(
                                attT_ps[:, t, :], att_sb[:, lt, t, :], ident
                            )
                        nc.vector.tensor_copy(
                            out=oatt[:, :, lt * 128 : (lt + 1) * 128],
                            in_=attT_ps,
                        )

                    for lt in range(4):
                        lts = slice(lt * 128, (lt + 1) * 128)
                        for g in range(2):
                            num_ps = p2ps.tile(
                                [128, 4, 128], F32, tag="num", bufs=2,
                                name="num_ps",
                            )
                            for j in range(4):
                                h = 4 * g + j
                                nc.tensor.matmul(
                                    num_ps[:, j, 0 : DH + 1],
                                    lhsT=qp_sb[:, h, lts],
                                    rhs=kvsks_sb[:, h, :],
                                    start=True, stop=True,
                                )
                            r_sb = p2s.tile([128, 4], F32, tag="recip")
                            nc.vector.reciprocal(r_sb, num_ps[:, :, DH])
                            # att[l, (t h2) d] for heads 4g..4g+3
                            att_v = att_sb[:, lt, 2 * g : 2 * g + 2, :].rearrange(
                                "p t (h d) -> p (t h) d", d=DH
                            )
                            nc.vector.tensor_tensor(
                                out=att_v,
                                in0=num_ps[:, :, 0:DH],
                                in1=r_sb.unsqueeze(-1).broadcast_to((128, 4, DH)),
                                op=mybir.AluOpType.mult,
                            )
                        # transposes trail the num pipeline by one l-tile so
                        # the PE never waits on the recip/mult chain
                        if lt >= 1:
                            do_transpose(lt - 1)
                    do_transpose(3)
                    prev = (ch, oatt)
                emit_out(prev)

    nc.compile()
    return nc


def _prep_inputs(query_input, source_input, Wq, Wk, Wv, Wo, proj):
    """Host-side shard + layout prep. Returns in_maps for 8 cores."""
    bf = ml_dtypes.bfloat16
    ratio = 1.0 / float(np.sqrt(M))
    projt_all = (ratio * proj.T).astype(bf)  # [DH, M]
    projt2_all = np.zeros((2 * DH, 2 * M), dtype=bf)  # block-diag pair form
    projt2_all[0:DH, 0:M] = projt_all
    projt2_all[DH : 2 * DH, M : 2 * M] = projt_all
    def chunked(x):
        # [L, HID] -> [NCH*128, KT*512]: per chunk, partition p holds the
        # contiguous [k, l] block -> 8KB DMA lines
        return np.ascontiguousarray(
            x.reshape(NCH, 512, KT, 128).transpose(0, 3, 2, 1)
        ).reshape(NCH * 128, KT * 512).astype(bf)

    in_maps = []
    for c in range(8):
        b, hg = c // 2, c % 2
        hs = slice(hg * HC, (hg + 1) * HC)
        wk_c = Wk[:, hs, :].reshape(HID, HC * DH)
        wv_c = Wv[:, hs, :].reshape(HID, HC * DH)
        in_maps.append(
            {
                "xsT": chunked(source_input[b]),
                "xqT": chunked(query_input[b]),
                "wkv": np.concatenate([wk_c, wv_c], axis=1).astype(bf),
                "wq": Wq[:, hs, :].reshape(HID, HC * DH).astype(bf),
                "wo": Wo[hs].reshape(HC * DH, HID).astype(bf),
                "projt": projt_all,
                "projt2": projt2_all,
            }
        )
    return in_maps


def kernel(query_input, source_input, Wq, Wk, Wv, Wo, proj, training=0):
    global last_exec_time_ns, last_trace_path
    query_input = np.asarray(query_input, dtype=np.float32)
    source_input = np.asarray(source_input, dtype=np.float32)
    Wq = np.asarray(Wq, dtype=np.float32)
    Wk = np.asarray(Wk, dtype=np.float32)
    Wv = np.asarray(Wv, dtype=np.float32)
    Wo = np.asarray(Wo, dtype=np.float32)
    proj = np.asarray(proj, dtype=np.float32)

    if "nc" not in _cache:
        _cache["nc"] = build_kernel()
    nc = _cache["nc"]

    in_maps = _prep_inputs(query_input, source_input, Wq, Wk, Wv, Wo, proj)

    trace = os.environ.get("KERNEL_TRACE", "0") == "1"
    kwargs = {}
    if trace:
        try:
            import profhook

            profhook.install()
            kwargs["trace"] = True
            kwargs["trace_cores"] = [0]
        except Exception:
            pass
    try:
        res = bass_utils.run_bass_kernel_spmd(
            nc, in_maps, core_ids=list(range(8)), **kwargs
        )
    except Exception:
        if not kwargs:
            raise
        # trace post-processing can fail transiently (empty NTFF capture);
        # retry without tracing — compute is unaffected
        res = bass_utils.run_bass_kernel_spmd(nc, in_maps, core_ids=list(range(8)))
    if trace:
        global last_res
        last_exec_time_ns = res.exec_time_ns
        last_res = res
        if res.instructions_and_trace is not None:
            last_trace_path = res.instructions_and_trace[1]
            globals()["last_insts"] = res.instructions_and_trace[0]

    out = np.empty((B, L, HID), dtype=np.float32)
    for b in range(B):
        acc = res.results[2 * b]["outT"] + res.results[2 * b + 1]["outT"]
        out[b] = acc.T
    return out



# revision 48
# speedup vs baseline: 1.0192x; 1.0192x over previous
"""FAVOR+ (Performer) attention kernel for Trainium2, 8 NeuronCores.

Problem: nn_Attention_4810363372688
  B=4, L=4096, HID=1024, H=16, DH=64, M=128, noncausal relu-kernel FAVOR+.

Sharding: core c handles batch b=c//2 and head-group hg=c%2 (8 heads).
Each core computes a partial output (its 8 heads' contribution to the
output projection) in transposed layout [HID, L]; the host sums the two
head-group partials per batch and transposes back. No collectives.

Structure (all per 512-token chunk, software-pipelined):
  pass 1: kT = Wk.T x (transposed), two-stage kp hop via a block-diagonal
          projT pair matmul (relu(x+eps) on ACT), v in [l, hd] layout,
          kv/ks accumulated per head into persistent PSUM banks.
  pass 2: qT, qp (zero-padded K=128 hops), num per (head, l-tile) in
          [l, d] layout with den as column 64 -> reciprocal is a
          per-partition scalar op, att = num * recip via one broadcasted
          DVE mult per 4-head bank, PE-transpose of att pair-tiles feeds
          the output projection. The out-projection of chunk c is emitted
          after chunk c+1's qt/qp so it never stalls at a chunk seam.

Host prep: x pre-chunked to [NCH*128, KT*512] bf16 (8KB contiguous DMA
lines), ratio folded into projT, block-diag / zero-padded projT variants.
"""

import os
import numpy as np
import ml_dtypes

import concourse.bass as bass
import concourse.mybir as mybir
import concourse.tile as tile
from concourse import bacc
from concourse import bass_utils
from concourse.masks import make_identity

B, L, HID, H, DH, M = 4, 4096, 1024, 16, 64, 128
EPS = 1e-3
HC = H // 2          # heads per core = 8
KT = HID // 128      # 8 contraction k-tiles
LT = L // 128        # 32 token tiles (pass 1)
NCH = L // 512       # 8 L-chunks (pass 2)

BF16 = mybir.dt.bfloat16
F32 = mybir.dt.float32

_cache = {}

# exec time of the most recent run (ns), when KERNEL_TRACE=1
last_exec_time_ns = None
last_trace_path = None
last_insts = None
last_res = None


def build_kernel():
    nc = bacc.Bacc("TRN2", target_bir_lowering=False, debug=False, num_devices=1)

    # x inputs are host-pre-chunked: row block ch*128..ch*128+127 holds, for
    # partition p, the KT*512 contiguous values [k, l] of chunk ch -> every
    # DMA line is 8KB contiguous.
    xsT = nc.dram_tensor("xsT", [NCH * 128, KT * 512], BF16, kind="ExternalInput").ap()
    xqT = nc.dram_tensor("xqT", [NCH * 128, KT * 512], BF16, kind="ExternalInput").ap()
    wkv = nc.dram_tensor("wkv", [HID, 2 * HC * DH], BF16, kind="ExternalInput").ap()
    wq = nc.dram_tensor("wq", [HID, HC * DH], BF16, kind="ExternalInput").ap()
    wo = nc.dram_tensor("wo", [HC * DH, HID], BF16, kind="ExternalInput").ap()
    projt = nc.dram_tensor("projt", [DH, M], BF16, kind="ExternalInput").ap()
    projt2 = nc.dram_tensor("projt2", [2 * DH, 2 * M], BF16, kind="ExternalInput").ap()
    outT = nc.dram_tensor("outT", [HID, L], F32, kind="ExternalOutput").ap()

    xsT_c = xsT.rearrange("(c p) (k f) -> c p k f", p=128, f=512)  # [8,128,8,512]
    xqT_c = xqT.rearrange("(c p) (k f) -> c p k f", p=128, f=512)
    wkv_r = wkv.rearrange("(k p) f -> p k f", p=128)     # [128, 8, 1024]
    wq_r = wq.rearrange("(k p) f -> p k f", p=128)       # [128, 8, 512]
    wo_r = wo.rearrange("(t p) o -> p t o", p=128)       # [128, 4, 1024]

    with tile.TileContext(nc) as tc:
        with tc.tile_pool(name="singles", bufs=1) as singles:
            # xs chunk 0 first, split across both DGE rings, so pass 1 can
            # start as early as possible; weights follow
            # interleave the chunk-0 x slices and wkv k-slices across both
            # DGE rings in k order: the first kT accumulation paces with the
            # arrivals instead of waiting for the full 2.5MB
            xs0_sb = singles.tile([128, KT, 512], BF16)
            wkv_sb = singles.tile([128, KT, 2 * HC * DH], BF16)
            # first wave: xs0 slices + just the kT m-tile-0 weight columns
            # (subtile deps let m-tile 0 start streaming ~2-3us earlier);
            # second wave: the remaining wk/wv columns
            for k in range(KT):
                a, b = (nc.sync, nc.scalar) if k % 2 == 0 else (nc.scalar, nc.sync)
                a.dma_start(out=xs0_sb[:, k, :], in_=xsT_c[0, :, k, :])
                b.dma_start(out=wkv_sb[:, k, 0:128], in_=wkv_r[:, k, 0:128])
            for k in range(KT):
                eng = nc.scalar if k % 2 == 0 else nc.sync
                eng.dma_start(out=wkv_sb[:, k, 128:1024], in_=wkv_r[:, k, 128:1024])
            # small tensors next (kp hop needs projt2 within the first chunk)
            # projt: zero-padded per half so qp matmuls run as uniform K=128
            # (dead rows contract against zeros) -> full-rate streaming;
            # projt2 is the block-diagonal pair form (for the kp hop)
            projt_sb = singles.tile([128, 2, M], BF16)
            nc.gpsimd.memset(projt_sb, 0.0)
            nc.scalar.dma_start(out=projt_sb[0:DH, 0, :], in_=projt)
            nc.scalar.dma_start(out=projt_sb[DH:128, 1, :], in_=projt)
            projt2_sb = singles.tile([128, 2 * M], BF16)
            nc.scalar.dma_start(out=projt2_sb, in_=projt2)
            # pass-2 weights queue behind wkv on the scalar ring; they are
            # not needed until pass 2
            wq_sb = singles.tile([128, KT, HC * DH], BF16)
            nc.scalar.dma_start(out=wq_sb, in_=wq_r)
            wo_sb = singles.tile([128, 4, HID], BF16)
            nc.scalar.dma_start(out=wo_sb, in_=wo_r)
            # prefetch pass-2 chunk 0 of xq behind the weights
            xq0_sb = singles.tile([128, KT, 512], BF16)
            nc.scalar.dma_start(out=xq0_sb, in_=xqT_c[0, :, :, :])
            kvsks_sb = singles.tile([128, HC, DH + 1], BF16)
            eps_bias = singles.tile([128, 1], F32)
            nc.vector.memset(eps_bias, EPS)



            # ---------------- pass 1: k -> kp, v -> kvs/ks accumulation ----
            # two-stage K: kT = Wk.T x (transposed, like qT), then the cheap
            # DH->M hop kp = relu(kT.T @ projT + eps') per head-pair via one
            # block-diagonal matmul (full 128-row weights -> FWL).
            with (
                tc.tile_pool(name="p1s", bufs=2) as p1s,
                tc.tile_pool(name="p1ps", bufs=2, space="PSUM") as p1ps,
                tc.tile_pool(name="kvps", bufs=1, space="PSUM") as kvps,
            ):
                kv_ps_lo = kvps.tile([128, 4 * (DH + 1)], F32, tag="kvlo")
                kv_ps_hi = kvps.tile([128, 4 * (DH + 1)], F32, tag="kvhi")

                for ch in range(NCH):
                    if ch == 0:
                        xs = xs0_sb
                    else:
                        xs = p1s.tile([128, KT, 512], BF16, tag="xs", bufs=3)
                        nc.sync.dma_start(out=xs, in_=xsT_c[ch, :, :, :])
                    # kT [hd, l] per m-tile, heads pair-stacked
                    kt_sb = p1s.tile([128, 4, 512], BF16, tag="kt")
                    for t in range(4):
                        kt_ps = p1ps.tile([128, 512], F32, tag="ktp", name="kt_ps")
                        for k in range(KT):
                            nc.tensor.matmul(
                                kt_ps,
                                lhsT=wkv_sb[:, k, t * 128 : (t + 1) * 128],
                                rhs=xs[:, k, :],
                                start=(k == 0), stop=(k == KT - 1),
                            )
                        nc.vector.tensor_copy(out=kt_sb[:, t, :], in_=kt_ps)
                    for lt in range(4):
                        lts = slice(lt * 128, (lt + 1) * 128)
                        v_ps = p1ps.tile([128, 512], F32, tag="vps", name="v_ps")
                        for k in range(KT):
                            nc.tensor.matmul(
                                v_ps,
                                lhsT=xs[:, k, lts],
                                rhs=wkv_sb[:, k, 512:1024],
                                start=(k == 0), stop=(k == KT - 1),
                            )
                        # v with a ones column appended per head: [128, 8, 65]
                        v_sb = p1s.tile([128, HC, DH + 1], BF16, tag="v", bufs=3)
                        nc.scalar.copy(
                            v_sb[:, :, 0:DH],
                            v_ps.rearrange("p (h d) -> p h d", h=HC),
                        )
                        nc.gpsimd.memset(v_sb[:, :, DH : DH + 1], 1.0)
                        # kp hop: per pair-bank, two pair matmuls with the
                        # block-diag projt2 -> [l, 2*M]; relu+eps evac
                        kp_sb = p1s.tile([128, HC, 128], BF16, tag="kp", bufs=3)
                        for gb in range(2):
                            kp_ps = p1ps.tile(
                                [128, 2, 256], F32, tag=f"kpp{gb}", bufs=1,
                                name="kp_ps",
                            )
                            for tp in range(2):
                                t = 2 * gb + tp
                                nc.tensor.matmul(
                                    kp_ps[:, tp, :],
                                    lhsT=kt_sb[:, t, lts],
                                    rhs=projt2_sb,
                                    start=True, stop=True,
                                )
                            # relu(x + eps) ~= relu(x) + eps (err <= eps on a
                            # ~0.04% sliver of inputs) -> single ACT op
                            nc.scalar.activation(
                                kp_sb[:, 4 * gb : 4 * gb + 4, :],
                                kp_ps.rearrange("p a (b m) -> p (a b) m", m=128),
                                mybir.ActivationFunctionType.Relu,
                                bias=eps_bias,
                            )
                        # One accumulation group per PSUM bank: start only on
                        # the very first MM touching the bank.
                        first = ch == 0 and lt == 0
                        last = ch == NCH - 1 and lt == 3
                        for h in range(HC):
                            ps = kv_ps_lo if h < 4 else kv_ps_hi
                            j = h % 4
                            nc.tensor.matmul(
                                ps[:, j * (DH + 1) : (j + 1) * (DH + 1)],
                                lhsT=kp_sb[:, h, :],
                                rhs=v_sb[:, h, :],
                                start=(first and j == 0),
                                stop=(last and j == 3),
                            )

                nc.scalar.copy(
                    kvsks_sb[:, 0:4, :],
                    kv_ps_lo.rearrange("p (h d) -> p h d", h=4),
                )
                nc.vector.tensor_copy(
                    out=kvsks_sb[:, 4:8, :],
                    in_=kv_ps_hi.rearrange("p (h d) -> p h d", h=4),
                )

            # ---------------- pass 2: q -> qp -> num/den -> att -> out ------
            # num is computed per (head, l-tile) in [l, d] layout (lhsT = qp
            # l-tile, rhs = kvs||ks), so den is column 64 -> reciprocal is a
            # per-partition scalar; att = num * recip(den) is one broadcasted
            # DVE mult per 4-head bank. att pair-tiles [l, 128] are then
            # PE-transposed to [hd, l] for the output projection.
            ident = singles.tile([128, 128], BF16)
            make_identity(nc, ident)
            with (
                tc.tile_pool(name="p2s", bufs=2) as p2s,
                tc.tile_pool(name="p2ps", bufs=2, space="PSUM") as p2ps,
            ):
                def emit_out(item):
                    och, oatt = item
                    osl = slice(och * 512, (och + 1) * 512)
                    for j in range(8):
                        out_ps = p2ps.tile(
                            [128, 512], F32, tag="out", bufs=3, name="out_ps"
                        )
                        for t in range(4):
                            nc.tensor.matmul(
                                out_ps,
                                lhsT=wo_sb[:, t, j * 128 : (j + 1) * 128],
                                rhs=oatt[:, t, :],
                                start=(t == 0), stop=(t == 3),
                            )
                        out_sb = p2s.tile(
                            [128, 512], F32, tag="outsb", bufs=4, name="out_sb"
                        )
                        # alternate evac engine and DMA ring by j: halves the
                        # ACT chain latency and drains the tail twice as fast
                        if j % 2 == 0:
                            nc.scalar.copy(out_sb, out_ps)
                            nc.scalar.dma_start(
                                out=outT[j * 128 : (j + 1) * 128, osl], in_=out_sb
                            )
                        else:
                            nc.vector.tensor_copy(out=out_sb, in_=out_ps)
                            nc.sync.dma_start(
                                out=outT[j * 128 : (j + 1) * 128, osl], in_=out_sb
                            )

                xq_next = xq0_sb
                prev = None
                for ch in range(NCH):
                    xq = xq_next
                    if ch + 1 < NCH:
                        xq_next = p2s.tile([128, KT, 512], BF16, tag="xq", bufs=3)
                        nc.sync.dma_start(out=xq_next, in_=xqT_c[ch + 1, :, :, :])
                    # qT [hd, l] per m-tile; heads (2m, 2m+1) stay stacked in
                    # the two partition halves
                    qt_sb = p2s.tile([128, 4, 512], BF16, tag="qt", bufs=3)
                    for m in range(4):
                        qt_ps = p2ps.tile([128, 512], F32, tag="proj", name="qt_ps")
                        for k in range(KT):
                            nc.tensor.matmul(
                                qt_ps,
                                lhsT=wq_sb[:, k, m * 128 : (m + 1) * 128],
                                rhs=xq[:, k, :],
                                start=(k == 0), stop=(k == KT - 1),
                            )
                        nc.scalar.copy(qt_sb[:, m, :], qt_ps)
                    # qp = max(ratio*(q@proj.T), 0) + eps  (ratio folded in
                    # projt); odd heads run at base-64 via row-tiling
                    qp_sb = p2s.tile([128, HC, 512], BF16, tag="qp", bufs=3)
                    for h in range(HC):
                        qp_ps = p2ps.tile([128, 512], F32, tag="proj", name="qp_ps")
                        nc.tensor.matmul(
                            qp_ps,
                            lhsT=projt_sb[:, h % 2, :],
                            rhs=qt_sb[:, h // 2, :],
                            start=True, stop=True,
                        )
                        nc.vector.tensor_scalar(
                            qp_sb[:, h, :], qp_ps, 0.0, EPS,
                            op0=mybir.AluOpType.max, op1=mybir.AluOpType.add,
                        )
                    # out-projection of the PREVIOUS chunk goes here: its
                    # attT evacs complete while this chunk's qt/qp run, so
                    # the out matmuls never stall the PE at a chunk seam.
                    if prev is not None:
                        emit_out(prev)
                    # num/den per l-tile: out [128 l, 65], den in col 64
                    att_sb = p2s.tile([128, 4, 4, 128], BF16, tag="att", bufs=3)
                    oatt = p2s.tile([128, 4, 512], BF16, tag="oatt", bufs=3)

                    def do_transpose(lt):
                        # att pair-tiles [l, 128] -> [hd, l] for the out proj
                        attT_ps = p2ps.tile(
                            [128, 4, 128], BF16, tag="attT", bufs=1,
                            name="attT_ps",
                        )
                        for t in range(4):
                            nc.tensor.transpose(
                                attT_ps[:, t, :], att_sb[:, lt, t, :], ident
                            )
                        nc.vector.tensor_copy(
                            out=oatt[:, :, lt * 128 : (lt + 1) * 128],
                            in_=attT_ps,
                        )

                    for lt in range(4):
                        lts = slice(lt * 128, (lt + 1) * 128)
                        for g in range(2):
                            num_ps = p2ps.tile(
                                [128, 4, 128], F32, tag="num", bufs=2,
                                name="num_ps",
                            )
                            for j in range(4):
                                h = 4 * g + j
                                nc.tensor.matmul(
                                    num_ps[:, j, 0 : DH + 1],
                                    lhsT=qp_sb[:, h, lts],
                                    rhs=kvsks_sb[:, h, :],
                                    start=True, stop=True,
                                )
                            r_sb = p2s.tile([128, 4], F32, tag="recip")
                            nc.vector.reciprocal(r_sb, num_ps[:, :, DH])
                            # att[l, (t h2) d] for heads 4g..4g+3
                            att_v = att_sb[:, lt, 2 * g : 2 * g + 2, :].rearrange(
                                "p t (h d) -> p (t h) d", d=DH
                            )
                            nc.vector.tensor_tensor(
                                out=att_v,
                                in0=num_ps[:, :, 0:DH],
                                in1=r_sb.unsqueeze(-1).broadcast_to((128, 4, DH)),
                                op=mybir.AluOpType.mult,
                            )
                        # transposes trail the num pipeline by one l-tile so
                        # the PE never waits on the recip/mult chain
                        if lt >= 1:
                            do_transpose(lt - 1)
                    do_transpose(3)
                    prev = (ch, oatt)
                emit_out(prev)

    nc.compile()
    return nc


def _prep_inputs(query_input, source_input, Wq, Wk, Wv, Wo, proj):
    """Host-side shard + layout prep. Returns in_maps for 8 cores."""
    bf = ml_dtypes.bfloat16
    ratio = 1.0 / float(np.sqrt(M))
    projt_all = (ratio * proj.T).astype(bf)  # [DH, M]
    projt2_all = np.zeros((2 * DH, 2 * M), dtype=bf)  # block-diag pair form
    projt2_all[0:DH, 0:M] = projt_all
    projt2_all[DH : 2 * DH, M : 2 * M] = projt_all
    def chunked(x):
        # [L, HID] -> [NCH*128, KT*512]: per chunk, partition p holds the
        # contiguous [k, l] block -> 8KB DMA lines
        return np.ascontiguousarray(
            x.reshape(NCH, 512, KT, 128).transpose(0, 3, 2, 1)
        ).reshape(NCH * 128, KT * 512).astype(bf)

    in_maps = []
    for c in range(8):
        b, hg = c // 2, c % 2
        hs = slice(hg * HC, (hg + 1) * HC)
        wk_c = Wk[:, hs, :].reshape(HID, HC * DH)
        wv_c = Wv[:, hs, :].reshape(HID, HC * DH)
        in_maps.append(
            {
                "xsT": chunked(source_input[b]),
                "xqT": chunked(query_input[b]),
                "wkv": np.concatenate([wk_c, wv_c], axis=1).astype(bf),
                "wq": Wq[:, hs, :].reshape(HID, HC * DH).astype(bf),
                "wo": Wo[hs].reshape(HC * DH, HID).astype(bf),
                "projt": projt_all,
                "projt2": projt2_all,
            }
        )
    return in_maps


def kernel(query_input, source_input, Wq, Wk, Wv, Wo, proj, training=0):
    global last_exec_time_ns, last_trace_path
    query_input = np.asarray(query_input, dtype=np.float32)
    source_input = np.asarray(source_input, dtype=np.float32)
    Wq = np.asarray(Wq, dtype=np.float32)
    Wk = np.asarray(Wk, dtype=np.float32)
    Wv = np.asarray(Wv, dtype=np.float32)
    Wo = np.asarray(Wo, dtype=np.float32)
    proj = np.asarray(proj, dtype=np.float32)

    if "nc" not in _cache:
        _cache["nc"] = build_kernel()
    nc = _cache["nc"]

    in_maps = _prep_inputs(query_input, source_input, Wq, Wk, Wv, Wo, proj)

    trace = os.environ.get("KERNEL_TRACE", "0") == "1"
    kwargs = {}
    if trace:
        try:
            import profhook

            profhook.install()
            kwargs["trace"] = True
            kwargs["trace_cores"] = [0]
        except Exception:
            pass
    try:
        res = bass_utils.run_bass_kernel_spmd(
            nc, in_maps, core_ids=list(range(8)), **kwargs
        )
    except Exception:
        if not kwargs:
            raise
        # trace post-processing can fail transiently (empty NTFF capture);
        # retry without tracing — compute is unaffected
        res = bass_utils.run_bass_kernel_spmd(nc, in_maps, core_ids=list(range(8)))
    if trace:
        global last_res
        last_exec_time_ns = res.exec_time_ns
        last_res = res
        if res.instructions_and_trace is not None:
            last_trace_path = res.instructions_and_trace[1]
            globals()["last_insts"] = res.instructions_and_trace[0]

    out = np.empty((B, L, HID), dtype=np.float32)
    for b in range(B):
        acc = res.results[2 * b]["outT"] + res.results[2 * b + 1]["outT"]
        out[b] = acc.T
    return out

